# revision 5
# baseline (speedup 1.0000x reference)
"""Trainium2 Bass kernel for the CapibaraByte recurrent-scan problem, v2.

Reference computation (B=128, T=1024, D_IN=256, H=2048):
    conv = einsum('btd,dh->bth', x, W_conv)
    step:  s <- 0.9*s + 0.1*gelu(s @ W_state + conv[:,t] + bias)
    out = (s @ W_state + bias, s)

Data-parallel over batch across 8 cores (B_local=16/core); scan fully
on-core.  Per-step GEMM is state-stationary with 4-way PE column tiling
(4 concurrent N=512 streams = the 8-XBUS peak, ~3.4us/step).

v2 changes vs v1:
 - The per-step [b,h] -> [h,b] transpose is done as 4 full 128x128 PE
   transposes of the *packed* PSUM layout (partitions 32g+b) instead of
   16 thin [16,128] transposes: state lives in a padded [128, 512]
   layout col = 128*c + 32*g + b  <->  h = 128*(4g+c) + p, b<16 valid.
 - Per-128-col-chunk software pipeline: evict(ACT) -> transpose(PE) ->
   +conv(DVE) -> gelu(ACT) -> blend(DVE) -> bf16(DVE), with the next
   step's matmuls ordered chunk-major so they start on chunk 0 of the
   new state while chunks 1-3 are still in flight.
 - 0.9*state prescale runs during the matmul phase (off critical path).
 - sW is evicted to bf16 (halves evict cost; product rounding ~0.4%).
"""

import sys

for _p in ("/opt/trn_rl_repo",):
    if _p not in sys.path:
        sys.path.insert(0, _p)

import numpy as np
import ml_dtypes

import concourse.bass as bass
import concourse.tile as tile
from concourse import bacc, mybir
from concourse.bass import ds
from concourse.bass_utils import run_bass_kernel_spmd

AFT = mybir.ActivationFunctionType
ALU = mybir.AluOpType
F32 = mybir.dt.float32
BF16 = mybir.dt.bfloat16

B, T_FULL, D_IN, H = 128, 1024, 256, 2048
NCORES = 8
BL = B // NCORES            # 16 batch rows per core
KT = H // 128               # 16 contraction tiles
MT = H // 128               # 16 output h-tiles
UPDATE = 0.1
NCH = 4                     # 128-col chunks of the padded state layout


def col0(tau):
    """Start column of h-tile tau in the padded [128, 512] state layout."""
    return 128 * (tau % 4) + 32 * (tau // 4)


def build(T_steps=T_FULL, U=8, act=AFT.Gelu_apprx_tanh, static_loop=False,
          with_bias=False):
    assert T_steps % U == 0
    nc = bacc.Bacc("TRN2", target_bir_lowering=False, debug=False,
                   num_devices=NCORES)

    # padded by 2 conv blocks so the steady-state prefetch of blocks
    # (2j+2, 2j+3) never reads out of range on the final body
    xT_d = nc.dram_tensor("xT", [2, 128, (T_steps + 2 * U) * BL], BF16,
                          kind="ExternalInput").ap()
    w_d = nc.dram_tensor("w_arr", [128, KT * H], BF16,
                         kind="ExternalInput").ap()
    wc_d = nc.dram_tensor("wc_arr", [128, 2 * H], BF16,
                          kind="ExternalInput").ap()
    biasT_d = nc.dram_tensor("bias_pad", [128, 512], F32,
                             kind="ExternalInput").ap()
    ident_d = nc.dram_tensor("ident", [128, 128], BF16,
                             kind="ExternalInput").ap()
    outT_d = nc.dram_tensor("outT", [128, 512], F32,
                            kind="ExternalOutput").ap()
    stT_d = nc.dram_tensor("stT", [128, 512], F32,
                           kind="ExternalOutput").ap()

    UB = U * BL  # conv block column count per k-tile
    # matmul k order, chunk-major: state chunk c feeds k-tiles {c, c+4, ...}
    korder = [c + 4 * j for c in range(NCH) for j in range(4)]

    with tile.TileContext(nc) as tc:
        with (
            tc.tile_pool(name="persist", bufs=1) as persist,
            tc.tile_pool(name="work", bufs=2) as work,
            tc.tile_pool(name="psum_su", bufs=2, space="PSUM") as psum_su,
            tc.tile_pool(name="psum_t", bufs=1, space="PSUM") as psum_t,
        ):
            # ---- resident tensors ----
            w_sb = persist.tile([128, KT * H], BF16, tag="w_sb")
            nc.sync.dma_start(w_sb[:], w_d[:])
            wc_sb = persist.tile([128, 2 * H], BF16, tag="wc_sb")
            nc.sync.dma_start(wc_sb[:], wc_d[:])
            biasT_sb = persist.tile([128, 512], F32, tag="biasT_sb")
            nc.sync.dma_start(biasT_sb[:], biasT_d[:])
            ident_sb = persist.tile([128, 128], BF16, tag="ident_sb")
            nc.sync.dma_start(ident_sb[:], ident_d[:])

            # state, padded layout, split per 128-col chunk so dependency
            # tracking is per-chunk: next step's chunk-c matmuls wait only on
            # chunk c's bf16 copy, not on all four.
            stbf = []
            st32 = []
            for c in range(NCH):
                sb = persist.tile([128, 128], BF16, tag=f"stbf{c}")
                nc.vector.memset(sb[:], 0.0)
                stbf.append(sb)
                s3 = persist.tile([128, 128], F32, tag=f"st32{c}")
                nc.vector.memset(s3[:], 0.0)
                st32.append(s3)

            def mm_phase(xt=None, u=0):
                """Col-tiled matmuls, chunk-major k order.  When xt is given,
                the conv projection x_t @ W_conv is fused in as 2 extra
                k-groups (stationary = x_t^T slice) accumulated into the same
                PSUM bank — first, since x is staged long before the state."""
                su = psum_su.tile([128, 512], F32, tag="su")
                if xt is not None:
                    for kc in range(2):
                        lhs = xt[:, kc * UB + u * BL:kc * UB + (u + 1) * BL]
                        for g in range(4):
                            nc.tensor.matmul(
                                su[32 * g:32 * g + BL, :],
                                lhsT=lhs,
                                rhs=wc_sb[:, kc * H + 512 * g:
                                          kc * H + 512 * (g + 1)],
                                start=(kc == 0), stop=False,
                                tile_position=(0, 32 * g),
                            )
                for j, k in enumerate(korder):
                    lhs = stbf[k % 4][:, 32 * (k // 4):32 * (k // 4) + BL]
                    for g in range(4):
                        nc.tensor.matmul(
                            su[32 * g:32 * g + BL, :],
                            lhsT=lhs,
                            rhs=w_sb[:, k * H + 512 * g:k * H + 512 * (g + 1)],
                            start=(xt is None and j == 0),
                            stop=(j == KT - 1),
                            tile_position=(0, 32 * g),
                        )
                return su

            def evict_transpose(su):
                """PSUM packed [32g+b, n] -> bf16 sbuf -> 4 PE transposes."""
                pts = []
                for c in range(NCH):
                    sbf = work.tile([128, 128], BF16, tag=f"subf{c}")
                    nc.scalar.copy(sbf[:], su[:, 128 * c:128 * (c + 1)])
                    pt = psum_t.tile([128, 128], BF16, tag=f"pt{c}",
                                     name=f"pt{c}")
                    nc.tensor.transpose(pt[:], sbf[:], ident_sb[:])
                    pts.append(pt)
                return pts

            def do_step(xt, u):
                # 0.9*state prescale, independent of this step's matmuls
                # (on ACT: DVE carries the evict+blend load this revision)
                tmps = []
                for c in range(NCH):
                    tmp = work.tile([128, 128], F32, tag=f"tmp{c}")
                    nc.scalar.mul(tmp[:], st32[c][:], 1.0 - UPDATE)
                    tmps.append(tmp)
                su = mm_phase(xt, u)
                # per-chunk interleaved chains on per-chunk tiles: chunk 0's
                # chain completes and unblocks next-step chunk-0 matmuls while
                # chunks 1-3 are still in flight.  conv is already inside su,
                # so without a bias the gelu reads the transposed PSUM
                # directly.  The blend writes the bf16 state (what the next
                # matmul needs) on the critical path; the f32 master is
                # maintained by a duplicate blend off the critical path
                # (same inputs -> numerically identical).
                # All 4 evicts batched on DVE before the chains: keeps the
                # ACT FIFO free for gelus (per-chunk interleaved emission put
                # evict_{c+1} behind gelu_c's wait on ACT, staggering every
                # chunk chain by a full gelu round-trip).
                sbfs = []
                for c in range(NCH):
                    sbf = work.tile([128, 128], BF16, tag=f"subf{c}")
                    nc.vector.tensor_copy(sbf[:], su[:, 128 * c:128 * (c + 1)])
                    sbfs.append(sbf)
                gcs = []
                for c in range(NCH):
                    sl = slice(128 * c, 128 * (c + 1))
                    sbf = sbfs[c]
                    pt = psum_t.tile([128, 128], BF16, tag=f"pt{c}",
                                     name=f"pt{c}")
                    nc.tensor.transpose(pt[:], sbf[:], ident_sb[:])
                    gc = work.tile([128, 128], F32, tag=f"gc{c}")
                    if with_bias:
                        uc = work.tile([128, 128], F32, tag=f"uc{c}")
                        nc.vector.tensor_tensor(uc[:], pt[:], biasT_sb[:, sl],
                                                ALU.add)
                        nc.scalar.activation(gc[:], uc[:], act)
                    else:
                        nc.scalar.activation(gc[:], pt[:], act)
                    nc.vector.scalar_tensor_tensor(
                        stbf[c][:], gc[:], UPDATE, tmps[c][:],
                        ALU.mult, ALU.add)
                    gcs.append(gc)
                for c in range(NCH):
                    nc.vector.scalar_tensor_tensor(
                        st32[c][:], gcs[c][:], UPDATE, tmps[c][:],
                        ALU.mult, ALU.add)

            # ping-pong x staging buffers (persistent so the prefetch DMA of
            # body j+1's blocks can be issued from inside body j)
            xA = persist.tile([128, 2 * UB], BF16, tag="xA")
            xB = persist.tile([128, 2 * UB], BF16, tag="xB")

            def load_x(xt, blk):
                """blk may be a python int or a loop-register expression."""
                for kc in range(2):
                    nc.sync.dma_start(
                        xt[:, kc * UB:(kc + 1) * UB],
                        xT_d[kc, :, ds(blk * UB, UB)])

            n_iters = T_steps // U
            assert n_iters % 2 == 0
            load_x(xA, 0)
            load_x(xB, 1)

            def body(i):
                for u in range(U):
                    do_step(xA, u)
                load_x(xA, 2 * i + 2)
                for u in range(U):
                    do_step(xB, u)
                load_x(xB, 2 * i + 3)

            if static_loop:
                for i in range(n_iters // 2):
                    body(i)
            else:
                with tc.For_i(0, n_iters // 2, 1,
                              hint_engines=(mybir.EngineType.PE,
                                            mybir.EngineType.DVE)) as i:
                    body(i)

            # ---- final output = state @ W_state + bias ----
            su = mm_phase()
            pts = evict_transpose(su)
            outf = work.tile([128, 512], F32, tag="outf")
            for c in range(NCH):
                sl = slice(128 * c, 128 * (c + 1))
                nc.vector.tensor_tensor(outf[:, sl], pts[c][:],
                                        biasT_sb[:, sl], ALU.add)
            nc.sync.dma_start(outT_d[:], outf[:])
            for c in range(NCH):
                nc.sync.dma_start(stT_d[:, 128 * c:128 * (c + 1)], st32[c][:])

    nc.compile()
    return nc


def host_inputs(x, W_state, W_conv, bias, T_steps=T_FULL):
    """Per-core input dicts. x: (B, T_steps, D_IN) f32."""
    bf = ml_dtypes.bfloat16
    w_arr = np.ascontiguousarray(
        W_state.reshape(KT, 128, H).transpose(1, 0, 2).reshape(128, KT * H)
    ).astype(bf)
    wc_arr = np.ascontiguousarray(
        W_conv.reshape(2, 128, H).transpose(1, 0, 2).reshape(128, 2 * H)
    ).astype(bf)
    # bias_pad[p, 128c+32g+j] = bias[128*(4g+c)+p]
    b16 = bias.reshape(16, 128).T.reshape(128, 4, 4)          # [p, g, c]
    bias_pad = np.ascontiguousarray(
        np.broadcast_to(b16.transpose(0, 2, 1)[:, :, :, None],
                        (128, 4, 4, 32)).reshape(128, 512)
    ).astype(np.float32)
    ident = np.eye(128, dtype=np.float32).astype(bf)

    in_maps = []
    U = 8
    for c in range(NCORES):
        xs = x[c * BL:(c + 1) * BL]          # [BL, T, D]
        xT = np.zeros((2, 128, (T_steps + 2 * U) * BL), dtype=bf)
        xT[:, :, :T_steps * BL] = (
            xs.reshape(BL, T_steps, 2, 128).transpose(2, 3, 1, 0)
            .reshape(2, 128, T_steps * BL).astype(bf))
        in_maps.append({
            "xT": xT, "w_arr": w_arr, "wc_arr": wc_arr,
            "bias_pad": bias_pad, "ident": ident,
        })
    return in_maps


def _unpad(arr):
    """[128, 512] padded -> [BL, H]: arr[p, 128c+32g+b] = val[b, 128*(4g+c)+p]."""
    return arr.reshape(128, 4, 4, 32).transpose(3, 2, 1, 0).reshape(
        32, H)[:BL]


def gather_outputs(results):
    out = np.empty((B, H), np.float32)
    st = np.empty((B, H), np.float32)
    for c, r in enumerate(results):
        out[c * BL:(c + 1) * BL] = _unpad(r["outT"])
        st[c * BL:(c + 1) * BL] = _unpad(r["stT"])
    return out, st


_NC_CACHE = {}


def _get_nc(T_steps=T_FULL, U=8, with_bias=False):
    key = (T_steps, U, with_bias)
    if key not in _NC_CACHE:
        _NC_CACHE[key] = build(T_steps, U, with_bias=with_bias)
    return _NC_CACHE[key]


def kernel(x, W_state, W_conv, bias):
    x = np.asarray(x, np.float32)
    W_state = np.asarray(W_state, np.float32)
    W_conv = np.asarray(W_conv, np.float32)
    bias = np.asarray(bias, np.float32)
    # Specialize the build: with a zero bias the gelu reads the transposed
    # PSUM directly (one fewer op + sync hop on the per-step critical path).
    nc = _get_nc(with_bias=bool(np.any(bias)))
    in_maps = host_inputs(x, W_state, W_conv, bias)
    res = run_bass_kernel_spmd(nc, in_maps, list(range(NCORES)))
    return gather_outputs(res.results)
